# revision 12
# baseline (speedup 1.0000x reference)
"""PerceiverAttention TRN2 kernel: B=8 batch elements, one per NeuronCore.

Per-core pipeline (all shapes hardcoded):
  latents [256,1024], context [4096,1024] -> out [256,1024]
  - LayerNorm both (gains folded into weights on host; biases folded into
    per-partition bias adds / final bias).
  - Q^T = Wq^T @ latnorm^T  (PE transpose of latnorm, then matmul)
  - per 512-row context chunk: LN -> PE transpose -> K^T/V projections,
    RoPE on K^T via signed-permutation matmul + cos/sin elementwise,
    per-head S^T = K_rot^T^T... S^T[s,n] = sum_d K_rot[d,s] Q^T[d,n],
    P = exp(0.125*S) (no max-subtraction; |scores|<~6), AV with a ones
    column appended to V to accumulate softmax denominators, accumulated
    into SBUF.
  - epilogue: normalize by reciprocal rowsums, out = O @ Wo + bo.
"""

import numpy as np
import ml_dtypes
from contextlib import ExitStack

import concourse.bass as bass
import concourse.bacc as bacc
import concourse.tile as tile
from concourse import mybir
from concourse.masks import make_identity

B, N, S, DIM, HEADS, DHEAD = 8, 256, 4096, 1024, 16, 64
INNER = 1024
EPS = 1e-5
SC = 512           # context rows per chunk
NCH = S // SC      # 8
FP32 = mybir.dt.float32
BF16 = mybir.dt.bfloat16
Alu = mybir.AluOpType
Act = mybir.ActivationFunctionType


def _ln_apply(nc, pool, raw, out_bf, eps_sb):
    """raw [128, D] fp32 -> out_bf [128, D] bf16 layernormed (no gain/bias)."""
    D = 1024
    stats = pool.tile([128, 2, 6], FP32)
    nc.vector.bn_stats(stats[:, 0, :], raw[:, 0:512])
    nc.vector.bn_stats(stats[:, 1, :], raw[:, 512:1024])
    mv = pool.tile([128, 2], FP32)
    nc.vector.bn_aggr(mv, stats)
    # mv[:,1] = rstd = 1/sqrt(var+eps)
    nc.scalar.activation(mv[:, 1:2], mv[:, 1:2], Act.Sqrt, bias=eps_sb)
    nc.vector.reciprocal(mv[:, 1:2], mv[:, 1:2])
    # Two single-AP-pointer Pool ops: instructions with two AP scalar
    # pointers plus two sem waits overflow the descriptor's sync-command
    # slots in walrus codegen ("Too many sync wait commands").
    nc.gpsimd.tensor_scalar(raw, raw, mv[:, 0:1], None, op0=Alu.subtract)
    nc.gpsimd.tensor_scalar(out_bf, raw, mv[:, 1:2], None, op0=Alu.mult)


def build_nc():
    # Bacc (not Bass): its finalize() runs generate_event_semaphores, which
    # legalizes multi-wait instructions (HW caps 1 sem wait per instruction).
    nc = bacc.Bacc()
    lat_e = nc.declare_dram_parameter("lat", [N, DIM], FP32, isOutput=False)
    ctx_e = nc.declare_dram_parameter("ctx", [S, DIM], FP32, isOutput=False)
    wq_e = nc.declare_dram_parameter("wq", [DIM, INNER], BF16, isOutput=False)
    wkv_e = nc.declare_dram_parameter("wkv", [DIM, 2 * INNER], BF16, isOutput=False)
    wo_e = nc.declare_dram_parameter("wo", [INNER, DIM], BF16, isOutput=False)
    cos_e = nc.declare_dram_parameter("cos128", [128, S], FP32, isOutput=False)
    sin_e = nc.declare_dram_parameter("sin128", [128, S], FP32, isOutput=False)
    sperm_e = nc.declare_dram_parameter("sperm", [128, 128], FP32, isOutput=False)
    qb_e = nc.declare_dram_parameter("qb", [128, 8], FP32, isOutput=False)
    kb_e = nc.declare_dram_parameter("kb", [128, 8], FP32, isOutput=False)
    bo_e = nc.declare_dram_parameter("bo", [DIM], FP32, isOutput=False)
    out_e = nc.declare_dram_parameter("out", [N, DIM], FP32, isOutput=True)
    rs_scratch = nc.dram_tensor("rs_scratch", [HEADS, N], FP32)

    with tile.TileContext(nc) as tc, ExitStack() as es:
        singles = es.enter_context(tc.tile_pool(name="singles", bufs=1))

        cos_sb = singles.tile([128, S], FP32)
        sin_sb = singles.tile([128, S], FP32)
        nc.gpsimd.dma_start(cos_sb[:], cos_e[:])
        nc.gpsimd.dma_start(sin_sb[:], sin_e[:])
        wkv_sb = singles.tile([128, 8, 2 * INNER], BF16)
        nc.gpsimd.dma_start(wkv_sb[:], wkv_e[:].rearrange("(t p) c -> p t c", p=128))
        wo_sb = singles.tile([128, 8, DIM], BF16)
        nc.gpsimd.dma_start(wo_sb[:], wo_e[:].rearrange("(t p) c -> p t c", p=128))
        sperm_sb = singles.tile([128, 128], FP32)
        nc.gpsimd.dma_start(sperm_sb[:], sperm_e[:])
        qb_sb = singles.tile([128, 8], FP32)
        nc.gpsimd.dma_start(qb_sb[:], qb_e[:])
        kb_sb = singles.tile([128, 8], FP32)
        nc.gpsimd.dma_start(kb_sb[:], kb_e[:])
        bo_sb = singles.tile([128, DIM], FP32)
        bo_bcast = bass.AP(tensor=bo_e.tensor if hasattr(bo_e, "tensor") else bo_e,
                           offset=bo_e[:].offset,
                           ap=[[0, 128]] + list(bo_e[:].ap))
        nc.gpsimd.dma_start(bo_sb[:], bo_bcast)

        ident = singles.tile([128, 128], BF16)
        make_identity(nc, ident)
        zb = singles.tile([128, 1], FP32)
        nc.vector.memset(zb, 0.0)
        eps_sb = singles.tile([128, 1], FP32)
        nc.vector.memset(eps_sb, EPS)

        qt_sb = singles.tile([128, 8, N], FP32)      # Q^T, tile t = inner dims [128t,128t+128)
        oacc = singles.tile([65, HEADS, N], FP32)    # row 64 = softmax denominators
        nc.vector.memset(oacc[:], 0.0)

        # ---------------- prologue: latents -> Q^T ----------------
        with tc.tile_pool(name="prolog", bufs=1) as pp, \
             tc.tile_pool(name="prolog_ps", bufs=2, space="PSUM") as ppp:
            wq_sb = pp.tile([128, 8, INNER], BF16)
            nc.gpsimd.dma_start(wq_sb[:], wq_e[:].rearrange("(t p) c -> p t c", p=128))
            lat_raw = pp.tile([128, 2, DIM], FP32)
            nc.sync.dma_start(lat_raw[:], lat_e[:].rearrange("(t p) d -> p t d", p=128))
            lat_std = pp.tile([128, 2, DIM], BF16)
            for t in range(2):
                _ln_apply(nc, pp, lat_raw[:, t, :], lat_std[:, t, :], eps_sb)
            latT = pp.tile([128, 8, N], BF16)
            for j in range(8):
                tp = ppp.tile([128, 2, 128], BF16)
                for i2 in range(2):
                    nc.tensor.transpose(tp[:, i2, :], lat_std[:, i2, j * 128:(j + 1) * 128], ident)
                nc.vector.tensor_copy(latT[:, j, :].rearrange("p (a b) -> p a b", a=2), tp[:])
            for mt in range(8):
                qp = ppp.tile([128, N], FP32)
                for k in range(8):
                    nc.tensor.matmul(qp[:], wq_sb[:, k, mt * 128:(mt + 1) * 128],
                                     latT[:, k, :], start=(k == 0), stop=(k == 7))
                nc.scalar.activation(qt_sb[:, mt, :], qp[:], Act.Identity,
                                     bias=qb_sb[:, mt:mt + 1])

        # ---------------- main loop over context chunks ----------------
        with tc.tile_pool(name="ctxp", bufs=3) as ctx_pool, \
             tc.tile_pool(name="stdp", bufs=3) as std_pool, \
             tc.tile_pool(name="stdT", bufs=2) as stdT_pool, \
             tc.tile_pool(name="ktr", bufs=1) as ktr_pool, \
             tc.tile_pool(name="krot", bufs=1) as krot_pool, \
             tc.tile_pool(name="vext", bufs=2) as vext_pool, \
             tc.tile_pool(name="ptp", bufs=2) as pt_pool, \
             tc.tile_pool(name="rtmp", bufs=2) as tmp_pool, \
             tc.tile_pool(name="ps_t", bufs=1, space="PSUM") as psum_t, \
             tc.tile_pool(name="ps_kv", bufs=2, space="PSUM") as psum_kv, \
             tc.tile_pool(name="ps_qk", bufs=2, space="PSUM") as psum_qk, \
             tc.tile_pool(name="ps_sw", bufs=1, space="PSUM") as psum_sw, \
             tc.tile_pool(name="ps_av", bufs=2, space="PSUM") as psum_av:
            for c in range(NCH):
                s0 = c * SC
                stdT = stdT_pool.tile([128, 8, SC], BF16)  # ctxnorm^T for this chunk
                for i in range(4):
                    raw = ctx_pool.tile([128, DIM], FP32)
                    nc.sync.dma_start(raw[:], ctx_e[s0 + i * 128: s0 + (i + 1) * 128, :])
                    std = std_pool.tile([128, DIM], BF16)
                    _ln_apply(nc, std_pool, raw[:], std[:], eps_sb)
                    tp = psum_t.tile([128, 8, 128], BF16)
                    for j in range(8):
                        nc.tensor.transpose(tp[:, j, :], std[:, j * 128:(j + 1) * 128], ident)
                    nc.vector.tensor_copy(stdT[:, :, i * 128:(i + 1) * 128], tp[:])

                # K^T projection (+bias), RoPE
                ktr = ktr_pool.tile([128, 8, SC], FP32)
                krot = krot_pool.tile([128, 8, SC], FP32)
                cs = cos_sb[:, s0:s0 + SC]
                sn = sin_sb[:, s0:s0 + SC]
                for j in range(8):
                    kp = psum_kv.tile([128, SC], FP32, tag="kv")
                    for k in range(8):
                        nc.tensor.matmul(kp[:], wkv_sb[:, k, j * 128:(j + 1) * 128],
                                         stdT[:, k, :], start=(k == 0), stop=(k == 7))
                    nc.scalar.activation(ktr[:, j, :], kp[:], Act.Identity,
                                         bias=kb_sb[:, j:j + 1])
                    sw = psum_sw.tile([128, SC], FP32)
                    nc.tensor.matmul(sw[:], sperm_sb[:], ktr[:, j, :])
                    tmp = tmp_pool.tile([128, SC], FP32)
                    nc.vector.tensor_mul(tmp[:], sw[:], sn)
                    nc.vector.tensor_mul(krot[:, j, :], ktr[:, j, :], cs)
                    nc.vector.tensor_add(krot[:, j, :], krot[:, j, :], tmp[:])

                # V projection -> V_ext with ones column
                vext = vext_pool.tile([128, 4, HEADS, DHEAD + 1], BF16)
                nc.gpsimd.memset(vext[:, :, :, DHEAD:DHEAD + 1], 1.0)
                for i in range(4):
                    for half in range(2):
                        vp = psum_kv.tile([128, 512], FP32, tag="kv")
                        for k in range(8):
                            nc.tensor.matmul(
                                vp[:], stdT[:, k, i * 128:(i + 1) * 128],
                                wkv_sb[:, k, INNER + half * 512: INNER + (half + 1) * 512],
                                start=(k == 0), stop=(k == 7))
                        nc.scalar.copy(vext[:, i, half * 8:(half + 1) * 8, 0:DHEAD],
                                       vp[:].rearrange("p (h d) -> p h d", h=8))

                # attention per head
                for h in range(HEADS):
                    t, base = h // 2, 64 * (h % 2)
                    pt = pt_pool.tile([128, 2, 512], BF16)
                    for pair in range(2):
                        sp = psum_qk.tile([128, 512], FP32)
                        for sl in range(2):
                            i = pair * 2 + sl
                            nc.tensor.matmul(
                                sp[:, sl * 256:(sl + 1) * 256],
                                krot[base:base + 64, t, i * 128:(i + 1) * 128],
                                qt_sb[base:base + 64, t, :])
                        nc.scalar.activation(pt[:, pair, :], sp[:], Act.Exp,
                                             bias=zb, scale=0.125)
                    av = psum_av.tile([65, N], FP32)
                    for i in range(4):
                        nc.tensor.matmul(av[:], vext[:, i, h, :],
                                         pt[:, i // 2, (i % 2) * 256:(i % 2 + 1) * 256],
                                         start=(i == 0), stop=(i == 3))
                    nc.vector.tensor_add(oacc[:, h, :], oacc[:, h, :], av[:])

        # ---------------- epilogue ----------------
        with tc.tile_pool(name="epi", bufs=1) as ep, \
             tc.tile_pool(name="epi_ps", bufs=2, space="PSUM") as epp:
            bc = ep.tile([64, HEADS, N], FP32)
            nc.vector.reciprocal(oacc[64:65, :, :], oacc[64:65, :, :])
            nc.sync.dma_start(rs_scratch[:], oacc[64:65, :, :])
            rsap = rs_scratch[:]
            bcast = bass.AP(tensor=rsap.tensor, offset=rsap.offset,
                            ap=[[0, 64]] + list(rsap.ap))
            nc.sync.dma_start(bc[:], bcast)
            for h in range(HEADS):
                nc.vector.tensor_mul(oacc[0:64, h, :], oacc[0:64, h, :], bc[:, h, :])
            obf = ep.tile([65, HEADS, N], BF16)
            nc.vector.tensor_copy(obf[:], oacc[:])
            ost = ep.tile([128, 8, N], BF16)
            for t in range(8):
                nc.sync.dma_start(ost[0:64, t, :], obf[0:64, 2 * t, :])
                nc.sync.dma_start(ost[64:128, t, :], obf[0:64, 2 * t + 1, :])
            osb = ep.tile([128, 2, DIM], FP32)
            for nsl in range(2):
                for half in range(2):
                    wop = epp.tile([128, 512], FP32)
                    for k in range(8):
                        nc.tensor.matmul(wop[:], ost[:, k, nsl * 128:(nsl + 1) * 128],
                                         wo_sb[:, k, half * 512:(half + 1) * 512],
                                         start=(k == 0), stop=(k == 7))
                    nc.vector.tensor_add(osb[:, nsl, half * 512:(half + 1) * 512],
                                         wop[:], bo_sb[:, half * 512:(half + 1) * 512])
            nc.sync.dma_start(out_e[:].rearrange("(t p) d -> p t d", p=128), osb[:])
    nc.finalize()
    return nc


def _host_prep(inputs):
    lat = np.asarray(inputs["latents"], np.float32)
    ctx = np.asarray(inputs["context"], np.float32)
    cos = np.asarray(inputs["cos"], np.float32)
    sin = np.asarray(inputs["sin"], np.float32)
    g_l = np.asarray(inputs["ln_lat_g"], np.float32)
    b_l = np.asarray(inputs["ln_lat_b"], np.float32)
    g_c = np.asarray(inputs["ln_ctx_g"], np.float32)
    b_c = np.asarray(inputs["ln_ctx_b"], np.float32)
    Wq = np.asarray(inputs["Wq"], np.float32)
    Wkv = np.asarray(inputs["Wkv"], np.float32)
    Wo = np.asarray(inputs["Wo"], np.float32)
    bo = np.asarray(inputs["bo"], np.float32)

    wq_bf = (g_l[:, None] * Wq).astype(ml_dtypes.bfloat16)
    wkv_bf = (g_c[:, None] * Wkv).astype(ml_dtypes.bfloat16)
    wo_bf = Wo.astype(ml_dtypes.bfloat16)
    qbias = (b_l @ Wq).astype(np.float32)
    kvbias = b_c @ Wkv
    kbias = kvbias[:INNER].astype(np.float32)
    vbias = kvbias[INNER:].astype(np.float32)
    bo_eff = (vbias @ Wo + bo).astype(np.float32)

    cosT = np.ascontiguousarray(cos.T)            # [32, S]
    sinT = np.ascontiguousarray(sin.T)
    cos128 = np.tile(cosT, (4, 1)).astype(np.float32)
    sgn = np.where((np.arange(128) % 64) < 32, -1.0, 1.0).astype(np.float32)
    sin128 = (np.tile(sinT, (4, 1)) * sgn[:, None]).astype(np.float32)
    d = np.arange(128)
    swap = np.where(d % 64 < 32, d + 32, d - 32)
    sperm = np.zeros((128, 128), np.float32)
    sperm[swap, d] = 1.0
    qb = np.ascontiguousarray(qbias.reshape(8, 128).T)
    kb = np.ascontiguousarray(kbias.reshape(8, 128).T)

    common = dict(wq=wq_bf, wkv=wkv_bf, wo=wo_bf, cos128=cos128, sin128=sin128,
                  sperm=sperm, qb=qb, kb=kb, bo=bo_eff)
    return [dict(lat=np.ascontiguousarray(lat[i]),
                 ctx=np.ascontiguousarray(ctx[i]), **common) for i in range(B)]


def kernel(**inputs) -> np.ndarray:
    from concourse import bass_utils
    in_maps = _host_prep(inputs)
    nc = build_nc()
    res = bass_utils.run_bass_kernel_spmd(nc, in_maps, list(range(B)))
    return np.stack([np.asarray(res.results[i]["out"], np.float32) for i in range(B)])


if __name__ == "__main__":
    nc = build_nc()
    print("built ok")
